# revision 9
# baseline (speedup 1.0000x reference)
"""Bahdanau attention pooling kernel for Trainium2 (8 NeuronCores, data parallel).

Problem shapes (hardcoded):
  encoder_features [256, 2048, 14, 14] f32, decoder_hidden [256, 512] f32,
  W_enc [2048, 512], b_enc [512], W_dec [512, 512], b_dec [512],
  W_full [512, 1], b_full [1].
Outputs: (context [256, 2048] f32, alpha [256, 196] f32).

Sharding: batch 256 -> 32 samples per core on 8 cores; weights replicated.
"""

import numpy as np

B_FULL = 256
N_CORES = 8
B_SH = B_FULL // N_CORES  # 32
PAIRS = B_SH // 2  # 16
C = 2048
P = 196  # 14*14 pixels
A = 512  # attention dim
D = 512  # decoder dim
KC = C // 128  # 16 C-chunks
MA = A // 128  # 4 A-chunks
KD = D // 128  # 4 D-chunks

_CACHE: dict = {}


def _build_nc(b_sh: int, n_cores: int = N_CORES):
    import concourse.bass as bass
    import concourse.bacc as bacc
    import concourse.tile as tile
    from concourse import mybir

    f32 = mybir.dt.float32
    f32r = mybir.dt.float32r
    bf16 = mybir.dt.bfloat16
    ts = bass.ts
    pairs = b_sh // 2

    nc = bacc.Bacc(
        "TRN2",
        target_bir_lowering=False,
        debug=False,
        num_devices=n_cores,
    )

    def inp(name, shape, dt):
        return nc.dram_tensor(name, shape, dt, kind="ExternalInput")

    enc = inp("enc", [pairs, 128, KC, 2, P], bf16)
    dech = inp("dech", [b_sh, D], f32)
    wenc = inp("wenc", [C, A], bf16)
    wdec = inp("wdec", [D, A], f32)
    wfull = inp("wfull", [128, MA], bf16)
    bsum = inp("bsum", [1, A], f32)
    ident = inp("ident", [32, 32], f32)
    ctx_o = nc.dram_tensor("ctx", [b_sh, 128, KC], f32, kind="ExternalOutput")
    alp_o = nc.dram_tensor("alpha", [b_sh, P], f32, kind="ExternalOutput")

    Relu = mybir.ActivationFunctionType.Relu
    Exp = mybir.ActivationFunctionType.Exp

    with tile.TileContext(nc) as tc:
        with tc.tile_pool(name="const", bufs=1) as constp:
            wenc_sb = constp.tile([128, KC, A], bf16)
            nc.sync.dma_start(wenc_sb[:], wenc.ap().rearrange("(k p) a -> p k a", p=128))
            wdec_sb = constp.tile([128, KD, A], f32)
            nc.sync.dma_start(wdec_sb[:], wdec.ap().rearrange("(k p) a -> p k a", p=128))
            wfull_sb = constp.tile([128, MA], bf16)
            nc.sync.dma_start(wfull_sb[:], wfull.ap())
            bsum_sb = constp.tile([1, A], f32)
            nc.sync.dma_start(bsum_sb[:], bsum.ap())
            ident_sb = constp.tile([32, 32], f32)
            nc.sync.dma_start(ident_sb[:], ident.ap())
            dech_sb = constp.tile([b_sh, D], f32)
            nc.sync.dma_start(dech_sb[:], dech.ap())
            ones32 = constp.tile([1, b_sh], f32)
            nc.vector.memset(ones32[:], 1.0)
            ones128 = constp.tile([1, 128], bf16)
            nc.vector.memset(ones128[:], 1.0)
            # dec_att.T with b_enc + b_dec folded in: [128, MA, b_sh]
            decatt_sb = constp.tile([128, MA, b_sh], f32)

            # --- setup: dec_att = decoder_hidden @ W_dec, transposed to [A, B] ---
            assert b_sh <= 32
            with tc.tile_pool(name="setup_ps", bufs=4, space="PSUM") as sps:
                dechT = constp.tile([128, KD, b_sh], f32)
                for kd in range(KD):
                    pt = sps.tile([128, b_sh], f32)
                    nc.tensor.transpose(
                        pt[:], dech_sb[0:b_sh, ts(kd, 128)], ident_sb[0:b_sh, 0:b_sh]
                    )
                    nc.scalar.copy(dechT[:, kd, :], pt[:])
                for m in range(MA):
                    dp = sps.tile([128, b_sh], f32)
                    for kd in range(KD):
                        nc.tensor.matmul(
                            dp[:],
                            wdec_sb[:, kd, ts(m, 128)],
                            dechT[:, kd, :],
                            start=(kd == 0),
                            stop=False,
                        )
                    nc.tensor.matmul(
                        dp[:], bsum_sb[0:1, ts(m, 128)], ones32[:], start=False, stop=True
                    )
                    nc.scalar.copy(decatt_sb[:, m, :], dp[:])

            # --- main loop over sample pairs ---
            with (
                tc.tile_pool(name="feats", bufs=3) as fp,
                tc.tile_pool(name="relu", bufs=2) as rp,
                tc.tile_pool(name="eps", bufs=4, space="PSUM") as eps,
                tc.tile_pool(name="aps", bufs=2, space="PSUM") as apsp,
                tc.tile_pool(name="bcps", bufs=2, space="PSUM") as bcps,
                tc.tile_pool(name="small", bufs=4) as sp,
                tc.tile_pool(name="bc", bufs=3) as bcp,
                tc.tile_pool(name="scr", bufs=2) as scp,
                tc.tile_pool(name="ctxp", bufs=3) as cxp,
            ):
                for pr in range(pairs):
                    ft = fp.tile([128, KC, 2, P], bf16)
                    nc.sync.dma_start(ft[:], enc.ap()[pr])
                    rl = rp.tile([128, MA, 2, P], bf16)
                    for m in range(MA):
                        ep = eps.tile([128, 2, P], f32)
                        for k in range(KC):
                            nc.tensor.matmul(
                                ep[:],
                                wenc_sb[:, k, ts(m, 128)],
                                ft[:, k, :, :],
                                start=(k == 0),
                                stop=(k == KC - 1),
                            )
                        for b2 in range(2):
                            nc.scalar.activation(
                                rl[:, m, b2, :],
                                ep[:, b2, :],
                                Relu,
                                bias=decatt_sb[:, m, bass.ds(2 * pr + b2, 1)],
                                scale=1.0,
                            )
                    ap_ps = apsp.tile([1, 2, P], f32)
                    for m in range(MA):
                        nc.tensor.matmul(
                            ap_ps[:],
                            wfull_sb[:, ts(m, 1)],
                            rl[:, m, :, :],
                            start=(m == 0),
                            stop=(m == MA - 1),
                        )
                    exp_row = sp.tile([1, 2, P], bf16)
                    nc.scalar.activation(exp_row[:], ap_ps[:], Exp)
                    for b2 in range(2):
                        b = 2 * pr + b2
                        bc_ps = bcps.tile([128, P], f32)
                        nc.tensor.matmul(
                            bc_ps[:], ones128[:], exp_row[0:1, b2, :],
                            start=True, stop=True,
                        )
                        ebc = bcp.tile([128, P], bf16)
                        nc.scalar.copy(ebc[:], bc_ps[:])
                        sume = sp.tile([128, 1], f32)
                        nc.vector.tensor_reduce(
                            sume[:], ebc[:], axis=mybir.AxisListType.X, op=mybir.AluOpType.add
                        )
                        rinv = sp.tile([128, 1], f32)
                        nc.vector.reciprocal(rinv[:], sume[:])
                        ctxt = cxp.tile([128, KC], f32)
                        for k in range(KC):
                            scr = scp.tile([128, P], bf16)
                            nc.vector.scalar_tensor_tensor(
                                out=scr[:],
                                in0=ft[:, k, b2, :],
                                scalar=1.0,
                                in1=ebc[:],
                                op0=mybir.AluOpType.mult,
                                op1=mybir.AluOpType.mult,
                                accum_out=ctxt[:, ts(k, 1)],
                            )
                        nc.vector.tensor_scalar_mul(ctxt[:], ctxt[:], rinv[:])
                        nc.sync.dma_start(ctx_o.ap()[b], ctxt[:])
                        alpha_row = sp.tile([1, P], f32)
                        nc.scalar.mul(alpha_row[:], exp_row[0:1, b2, :], rinv[0:1, 0:1])
                        nc.sync.dma_start(alp_o.ap()[b], alpha_row[:])

    nc.compile()
    return nc


def _prep_shared(W_enc, b_enc, W_dec, b_dec, W_full):
    import ml_dtypes

    wenc = np.ascontiguousarray(np.asarray(W_enc, dtype=np.float32)).astype(ml_dtypes.bfloat16)
    wdec = np.ascontiguousarray(np.asarray(W_dec, dtype=np.float32))
    wfull = np.ascontiguousarray(
        np.asarray(W_full, dtype=np.float32)[:, 0].reshape(MA, 128).T
    ).astype(ml_dtypes.bfloat16)
    bsum = (np.asarray(b_enc, dtype=np.float32) + np.asarray(b_dec, dtype=np.float32)).reshape(1, A)
    ident = np.eye(32, dtype=np.float32)
    return wenc, wdec, wfull, bsum, ident


def _prep_enc_shard(enc_shard):
    # [b_sh, C, P] -> [pairs, 128, KC, 2, P] in bf16
    import ml_dtypes

    b_sh = enc_shard.shape[0]
    arr = enc_shard.reshape(b_sh // 2, 2, KC, 128, P).transpose(0, 3, 2, 1, 4)
    return np.ascontiguousarray(arr).astype(ml_dtypes.bfloat16)


def kernel(encoder_features, decoder_hidden, W_enc, b_enc, W_dec, b_dec, W_full, b_full):
    from concourse.bass_utils import run_bass_kernel_spmd

    if "nc" not in _CACHE:
        _CACHE["nc"] = _build_nc(B_SH)
    nc = _CACHE["nc"]

    enc_full = np.asarray(encoder_features, dtype=np.float32).reshape(B_FULL, C, P)
    dech_full = np.asarray(decoder_hidden, dtype=np.float32)
    wenc, wdec, wfull, bsum, ident = _prep_shared(W_enc, b_enc, W_dec, b_dec, W_full)

    in_maps = []
    for i in range(N_CORES):
        sl = slice(i * B_SH, (i + 1) * B_SH)
        in_maps.append(
            {
                "enc": _prep_enc_shard(enc_full[sl]),
                "dech": np.ascontiguousarray(dech_full[sl]),
                "wenc": wenc,
                "wdec": wdec,
                "wfull": wfull,
                "bsum": bsum,
                "ident": ident,
            }
        )

    res = run_bass_kernel_spmd(nc, in_maps, list(range(N_CORES)))

    context = np.empty((B_FULL, C), dtype=np.float32)
    alpha = np.empty((B_FULL, P), dtype=np.float32)
    for i in range(N_CORES):
        sl = slice(i * B_SH, (i + 1) * B_SH)
        ctx_i = res.results[i]["ctx"]  # [B_SH, 128, KC]
        context[sl] = ctx_i.transpose(0, 2, 1).reshape(B_SH, C)
        alpha[sl] = res.results[i]["alpha"]
    return (context, alpha)
